# revision 14
# baseline (speedup 1.0000x reference)
"""Trainium2 Bass kernel for nn_EndToEndModel_86552180949694 (moe_routing).

RoBERTa-like 4-layer encoder run twice per sample batch:
  pass 1 (no LoRA) -> CLS -> router MLP -> argmax task id
  pass 2 (per-sample LoRA on q/v) -> CLS -> per-task classification head
Returns (router_logits [B,3], batch_logits [B,2]).

Sharding: pure data-parallel over batch, 4 samples per core on 8 cores.
All weights replicated; zero cross-core communication.  MoE dispatch is
realized densely: LoRA / head computed for all T=3 tasks and combined with
an argmax one-hot mask (rank-16 adapters make this ~8% extra FLOPs).

Layouts per core (P=128, S=128 tokens/sample, 4 samples = 4 "token tiles"):
  token-major  [128 tokens, H]  - residual stream, LN over free dim
  feature-major [H rows, 512 tokens] ("xT") - matmul operand layout
Matmul convention: out[M,N] = lhsT[K,M].T @ rhs[K,N], K = partitions.
"""

import numpy as np
import ml_dtypes

# ----------------------------------------------------------------------------
# Real model dimensions (hardcoded; kernel.py must be self-contained)
# ----------------------------------------------------------------------------
REAL_CFG = dict(
    B=32, S=128, V=50265, H=768, L=4, NH=12, DH=64, FF=3072, T=3, R=16,
    n_cores=8, lora_scale=2.0, eps=1e-5,
)

P = 128


def _install_tilefix():
    """Walrus in this container only allows 1 sync-wait on an SP Drain;
    Tile's tail drain attaches one wait per DMA queue.  Split them across
    several single-wait Drain instructions."""
    import concourse.mybir as mybir
    import concourse.tile as tile_mod
    from concourse.vector_clock import ScopedClock

    if getattr(tile_mod.TileContext, "_drain_split_installed", False):
        return

    def _drain_and_barrier(self, tick_clock, wait_clock):
        nc = self.nc
        drain_inst = nc.sync.drain()
        wait_clock.add_sem_waits(
            drain_inst.ins, ScopedClock({None: tick_clock.global_clock})
        )
        si = drain_inst.ins.sync_info
        if si is not None and si.on_wait and len(si.on_wait) > 1:
            waits = list(si.on_wait)
            drain_inst.ins.sync_info = mybir.SyncInfo(
                on_wait=[waits[0]], on_update=list(si.on_update)
            )
            for w in waits[1:]:
                d2 = nc.sync.drain()
                d2.ins.sync_info = mybir.SyncInfo(on_wait=[w], on_update=[])
        nc.all_engine_barrier()
        assert self.sems is not None
        popped = nc._tile_sem_poison_stack.pop()
        assert popped is self._sem_poison
        nc.clear_and_free_semaphores(list(self.sems.allocated().values()))
        nc.all_engine_barrier()

    tile_mod.TileContext._drain_and_barrier = _drain_and_barrier

    # Generalized: ANY instruction with >1 sync wait gets its extra waits
    # peeled onto single-wait NoOp carriers inserted just before it (same
    # engine, so sequential semantics are preserved).
    import bass_rust
    _orig_add = tile_mod.TileContext._add_instruction

    def _add_instruction_split(self, inst):
        si = getattr(inst, "sync_info", None)
        if si is not None and si.on_wait is not None and len(si.on_wait) > 1:
            waits = list(si.on_wait)
            eng = inst.engine
            if eng is not None and eng != mybir.EngineType.Unassigned:
                for w in waits[:-1]:
                    nop = bass_rust.InstNoOp(
                        name=f"waitnop-{self.nc.next_id()}", ins=[], outs=[])
                    nop.engine = eng
                    nop.sync_info = mybir.SyncInfo(on_wait=[w], on_update=[])
                    _orig_add(self, nop)
                inst.sync_info = mybir.SyncInfo(
                    on_wait=[waits[-1]], on_update=list(si.on_update))
        _orig_add(self, inst)

    tile_mod.TileContext._add_instruction = _add_instruction_split
    tile_mod.TileContext._drain_split_installed = True


# ----------------------------------------------------------------------------
# Host-side preprocessing
# ----------------------------------------------------------------------------
def _prep_host(inputs, cfg, mm_np):
    """Cast/transform weights, detect trivial biases / identity LN."""
    f32 = np.float32
    H, L, T, R = cfg["H"], cfg["L"], cfg["T"], cfg["R"]

    def az(x):  # all-zero
        return bool(np.all(np.asarray(x) == 0))

    def ao(x):  # all-one
        return bool(np.all(np.asarray(x) == 1))

    g = {k: np.asarray(v) for k, v in inputs.items()}

    host = {}
    host["word_emb"] = g["word_emb"].astype(f32)
    host["pos_emb"] = g["pos_emb"].astype(f32)
    for nm in ("Wq", "Wk", "Wv", "Wo"):
        host[nm.lower()] = np.ascontiguousarray(g[nm].astype(mm_np))
    host["w1"] = np.ascontiguousarray(g["W1"].astype(mm_np))
    host["w2"] = np.ascontiguousarray(g["W2"].astype(mm_np))
    # LoRA: A [T,L,R,H] -> aT [L,T,H,R]; B [T,L,H,R] -> bT [L,T,R,H] (x scale)
    sc = cfg["lora_scale"]
    host["aqT"] = np.ascontiguousarray(np.transpose(g["Aq"], (1, 0, 3, 2)).astype(mm_np))
    host["avT"] = np.ascontiguousarray(np.transpose(g["Av"], (1, 0, 3, 2)).astype(mm_np))
    host["bqT"] = np.ascontiguousarray((sc * np.transpose(g["Bq"], (1, 0, 3, 2))).astype(mm_np))
    host["bvT"] = np.ascontiguousarray((sc * np.transpose(g["Bv"], (1, 0, 3, 2))).astype(mm_np))
    # router (fold 1/max(temp, .05) into fc2)
    scale_r = 1.0 / max(float(np.asarray(g["r_temp"]).reshape(-1)[0]), 0.05)
    host["rfc1"] = np.ascontiguousarray(g["r_fc1_W"].astype(mm_np))
    host["rfc2"] = np.ascontiguousarray((g["r_fc2_W"] * scale_r).astype(mm_np))
    host["rfc1_b"] = g["r_fc1_b"].astype(f32)
    host["rfc2_b"] = (g["r_fc2_b"] * scale_r).astype(f32)
    # heads
    host["wd"] = np.ascontiguousarray(g["cls_dense_W"].astype(mm_np))
    host["wc"] = np.ascontiguousarray(g["cls_out_W"].astype(mm_np))
    host["bd"] = g["cls_dense_b"].astype(f32)
    host["bc"] = g["cls_out_b"].astype(f32)
    # biases / LN params (fp32)
    for nm in ("bq", "bk", "bv", "bo", "b1", "b2"):
        host[nm] = g[nm].astype(f32)
    for nm in ("emb_ln_g", "emb_ln_b", "ln1_g", "ln1_b", "ln2_g", "ln2_b"):
        host[nm] = g[nm].astype(f32)

    flags = dict(
        zb_q=az(g["bq"]), zb_k=az(g["bk"]), zb_v=az(g["bv"]), zb_o=az(g["bo"]),
        zb_1=az(g["b1"]), zb_2=az(g["b2"]),
        zb_r1=az(g["r_fc1_b"]), zb_r2=az(host["rfc2_b"]),
        zb_d=az(g["cls_dense_b"]), zb_c=az(g["cls_out_b"]),
        id_emb_ln=ao(g["emb_ln_g"]) and az(g["emb_ln_b"]),
        id_ln1=ao(g["ln1_g"]) and az(g["ln1_b"]),
        id_ln2=ao(g["ln2_g"]) and az(g["ln2_b"]),
        mask_ones=ao(g["attention_mask"]),
    )
    return host, flags


# ----------------------------------------------------------------------------
# Device program builder
# ----------------------------------------------------------------------------
def _build(cfg, flags, mm_dt_name="bfloat16"):
    import concourse.bass as bass
    import concourse.mybir as mybir
    from concourse.tile import TileContext
    from concourse.masks import make_identity

    _install_tilefix()

    f32 = mybir.dt.float32
    i32 = mybir.dt.int32
    mdt = getattr(mybir.dt, mm_dt_name)
    AF = mybir.ActivationFunctionType
    OP = mybir.AluOpType
    X = mybir.AxisListType.X

    S, H, L, NH, DH, FF, T, R, V = (cfg[k] for k in
                                    ("S", "H", "L", "NH", "DH", "FF", "T", "R", "V"))
    NT = cfg["B"] // cfg["n_cores"]
    NTOK = NT * S
    nH, nF = H // P, FF // P
    assert S == P and DH == 64 and H % P == 0 and FF % P == 0
    eps = cfg["eps"]
    inv_sqrt_dh = 1.0 / float(np.sqrt(DH))

    nc = bass.Bass("TRN2", target_bir_lowering=False, debug=False,
                   num_devices=cfg["n_cores"])

    d = {}
    def din(name, shape, dt):
        d[name] = nc.dram_tensor(name, list(shape), dt, kind="ExternalInput")
        return d[name]

    din("ids", (NT, S), i32)
    din("word_emb", (V, H), f32)
    din("pos_emb", (S, H), f32)
    for nm in ("wq", "wk", "wv", "wo"):
        din(nm, (L, H, H), mdt)
    din("w1", (L, H, FF), mdt)
    din("w2", (L, FF, H), mdt)
    din("aqT", (L, T, H, R), mdt)
    din("avT", (L, T, H, R), mdt)
    din("bqT", (L, T, R, H), mdt)
    din("bvT", (L, T, R, H), mdt)
    din("rfc1", (H, H), mdt)
    din("rfc2", (H, T), mdt)
    din("wd", (T, H, H), mdt)
    din("wc", (T, H, 2), mdt)
    if not flags["zb_q"]: din("bq", (L, H), f32)
    if not flags["zb_k"]: din("bk", (L, H), f32)
    if not flags["zb_v"]: din("bv", (L, H), f32)
    if not flags["zb_o"]: din("bo", (L, H), f32)
    if not flags["zb_1"]: din("b1", (L, FF), f32)
    if not flags["zb_2"]: din("b2", (L, H), f32)
    if not flags["zb_r1"]: din("rfc1_b", (H,), f32)
    if not flags["zb_r2"]: din("rfc2_b", (T,), f32)
    if not flags["zb_d"]: din("bd", (T, H), f32)
    if not flags["zb_c"]: din("bc", (T, 2), f32)
    if not flags["id_emb_ln"]:
        din("emb_ln_g", (H,), f32); din("emb_ln_b", (H,), f32)
    if not flags["id_ln1"]:
        din("ln1_g", (L, H), f32); din("ln1_b", (L, H), f32)
    if not flags["id_ln2"]:
        din("ln2_g", (L, H), f32); din("ln2_b", (L, H), f32)
    if not flags["mask_ones"]:
        din("attn_bias", (NT, S), f32)

    out_router = nc.dram_tensor("router_logits", [NT, T], f32, kind="ExternalOutput")
    out_batch = nc.dram_tensor("batch_logits", [NT, 2], f32, kind="ExternalOutput")

    # N-split for token-major matmul outputs of width H (fp32 PSUM bank = 512)
    nsplits = [(i, min(i + 512, H)) for i in range(0, H, 512)]

    with TileContext(nc) as tc:
        import contextlib
        ctx = contextlib.ExitStack()
        with ctx:
            cpool = ctx.enter_context(tc.tile_pool(name="const", bufs=1))
            wk = ctx.enter_context(tc.tile_pool(name="work", bufs=1))
            pp = ctx.enter_context(tc.tile_pool(name="ps", bufs=1, space="PSUM"))

            # ---------------- constants ----------------
            ident_f = cpool.tile([P, P], f32, name="ident_f")
            make_identity(nc, ident_f[:])
            ident_m = cpool.tile([P, P], mdt, name="ident_m")
            make_identity(nc, ident_m[:])
            pos_t = cpool.tile([P, H], f32, name="pos_t")
            nc.sync.dma_start(out=pos_t[:], in_=d["pos_emb"][:, :])
            eps_t = cpool.tile([P, 1], f32, name="eps_t")
            nc.gpsimd.memset(eps_t[:], eps)
            ssel = cpool.tile([NT, NTOK], f32, name="ssel")
            nc.gpsimd.memset(ssel[:], 0.0)
            nc.gpsimd.affine_select(
                out=ssel[:], in_=ssel[:], compare_op=OP.not_equal, fill=1.0,
                base=0, pattern=[[-1, NT], [0, S]], channel_multiplier=1,
            )

            # LN gamma/beta broadcast tiles (only when non-identity)
            ln_gb = {}
            def bcast_row(name, src_ap, width=None):
                w = H if width is None else width
                t_b = cpool.tile([P, w], f32, name=name + "_b")
                nc.sync.dma_start(out=t_b[:], in_=src_ap.to_broadcast([P, w]))
                return t_b
            if not flags["id_emb_ln"]:
                ln_gb["emb"] = (bcast_row("embg", d["emb_ln_g"][None, :]),
                                bcast_row("embb", d["emb_ln_b"][None, :]))
            if not flags["id_ln1"]:
                ln_gb["ln1"] = [(bcast_row(f"l1g{l}", d["ln1_g"][l, None, :]),
                                 bcast_row(f"l1b{l}", d["ln1_b"][l, None, :]))
                                for l in range(L)]
            if not flags["id_ln2"]:
                ln_gb["ln2"] = [(bcast_row(f"l2g{l}", d["ln2_g"][l, None, :]),
                                 bcast_row(f"l2b{l}", d["ln2_b"][l, None, :]))
                                for l in range(L)]
            b2_b = bo_b = None
            if not flags["zb_2"]:
                b2_b = [bcast_row(f"b2_{l}", d["b2"][l, None, :]) for l in range(L)]
            if not flags["zb_o"]:
                bo_b = [bcast_row(f"bo_{l}", d["bo"][l, None, :]) for l in range(L)]

            # attention-mask additive bias (rare path)
            bias_bc = None
            if not flags["mask_ones"]:
                bias_bc = []
                for s in range(NT):
                    mb = cpool.tile([P, S], f32, name=f"mb{s}")
                    nc.sync.dma_start(
                        out=mb[:],
                        in_=d["attn_bias"][s, None, :].to_broadcast([P, S]))
                    bias_bc.append(mb)

            # ---------------- small helpers ----------------
            def sm(tag):
                return wk.tile([P, 1], f32, tag=tag, bufs=6, name=tag)

            def layernorm(y_ap, x_out_ap, gb):
                """token-major LN over free dim H: x_out = (y-m)/sqrt(var+eps)*g+b
                y_ap may be PSUM or SBUF fp32."""
                sums = sm("ln_sum"); sqs = sm("ln_sq")
                nc.vector.reduce_sum(out=sums[:], in_=y_ap, axis=X)
                junk = wk.tile([P, H], f32, tag="scr768", bufs=2, name="ln_junk")
                nc.scalar.activation(junk[:], y_ap, AF.Square, accum_out=sqs[:])
                m = sm("ln_m"); ex2 = sm("ln_e2"); var = sm("ln_v")
                nc.vector.tensor_scalar_mul(m[:], sums[:], 1.0 / H)
                nc.vector.tensor_scalar_mul(ex2[:], sqs[:], 1.0 / H)
                nc.vector.tensor_tensor(out=var[:], in0=m[:], in1=m[:], op=OP.mult)
                nc.vector.tensor_tensor(out=var[:], in0=ex2[:], in1=var[:], op=OP.subtract)
                std = sm("ln_std"); rstd = sm("ln_rs"); negmr = sm("ln_nm")
                nc.scalar.activation(std[:], var[:], AF.Sqrt, bias=eps_t[:, :1])
                nc.vector.reciprocal(rstd[:], std[:])
                nc.vector.tensor_scalar(out=negmr[:], in0=m[:], scalar1=rstd[:, :1],
                                        scalar2=-1.0, op0=OP.mult, op1=OP.mult)
                nc.vector.tensor_scalar(out=x_out_ap, in0=y_ap, scalar1=rstd[:, :1],
                                        scalar2=negmr[:, :1], op0=OP.mult, op1=OP.add)
                if gb is not None:
                    gt, bt = gb
                    nc.vector.tensor_tensor(out=x_out_ap, in0=x_out_ap, in1=gt[:], op=OP.mult)
                    nc.vector.tensor_tensor(out=x_out_ap, in0=x_out_ap, in1=bt[:], op=OP.add)

            def transpose_to_fm(x_tiles, tag):
                """token-major fp32 NT tiles [P,H] -> feature-major mdt tiles
                nH x [P, NTOK]."""
                outs = []
                for c in range(nH):
                    ps = pp.tile([P, NTOK], f32, tag="ps512", bufs=2, name=f"pt{c}")
                    for t in range(NT):
                        nc.tensor.transpose(ps[:, t * S:(t + 1) * S],
                                            x_tiles[t][:, c * P:(c + 1) * P],
                                            ident_f[:])
                    xt = wk.tile([P, NTOK], mdt, tag=tag, bufs=nH + 2, name=f"{tag}{c}")
                    nc.scalar.copy(xt[:], ps[:])
                    outs.append(xt)
                return outs

            def bias_col(dram_ap, tag):
                """DMA a [128] bias slice to a [128,1] fp32 SBUF tile."""
                bt = wk.tile([P, 1], f32, tag=tag, bufs=4, name=tag)
                nc.sync.dma_start(out=bt[:], in_=dram_ap)
                return bt

            # ---------------- embedding (run once per pass) ----------------
            def embed():
                x0 = []
                for t in range(NT):
                    ids_t = wk.tile([P, 1], i32, tag="ids", bufs=NT, name=f"ids{t}")
                    nc.sync.dma_start(out=ids_t[:], in_=d["ids"][t, :, None])
                    emb = wk.tile([P, H], f32, tag="scr768", bufs=2, name=f"emb{t}")
                    nc.gpsimd.indirect_dma_start(
                        out=emb[:], out_offset=None, in_=d["word_emb"][:, :],
                        in_offset=bass.IndirectOffsetOnAxis(ap=ids_t[:, :1], axis=0),
                    )
                    nc.vector.tensor_tensor(out=emb[:], in0=emb[:], in1=pos_t[:],
                                            op=OP.add)
                    xt = wk.tile([P, H], f32, tag="xw", bufs=NT + 1, name=f"x0_{t}")
                    layernorm(emb[:], xt[:], ln_gb.get("emb"))
                    x0.append(xt)
                return x0

            # ---------------- encoder layer ----------------
            def layer(l, x_in, xT_in, lora_ctx):
                """One encoder layer.  x_in: NT token-major fp32 tiles.
                xT_in: nH feature-major mdt tiles.  Returns (x_out, xT_out_none)."""
                # ---- weight chunk DMAs ----
                wq_sb = []; wk_sb = []; wv_sb = []; wo_sb = []
                for nm, lst in (("wq", wq_sb), ("wk", wk_sb),
                                ("wv", wv_sb), ("wo", wo_sb)):
                    for c in range(nH):
                        wt = wk.tile([P, H], mdt, tag="w_enc", bufs=nH + 4,
                                     name=f"{nm}{l}_{c}")
                        nc.sync.dma_start(out=wt[:], in_=d[nm][l, c * P:(c + 1) * P, :])
                        lst.append(wt)

                # ---- LoRA u vectors (pass 2) ----
                uq = uv = None
                if lora_ctx is not None:
                    masks = lora_ctx
                    uq, uv = [], []
                    for nmA, ulist in (("aqT", uq), ("avT", uv)):
                        for t3 in range(T):
                            a_sb = wk.tile([P, nH * R], mdt, tag="a_sb", bufs=4,
                                           name=f"{nmA}{l}_{t3}")
                            nc.sync.dma_start(
                                out=a_sb[:].rearrange("p (c r) -> p c r", r=R),
                                in_=d[nmA][l, t3].rearrange("(c p) r -> p c r", p=P))
                            psu = pp.tile([P, NTOK], f32, tag="ps512", bufs=2,
                                          name=f"psu{t3}")
                            for c in range(nH):
                                nc.tensor.matmul(
                                    psu[0:R, :], lhsT=a_sb[:, c * R:(c + 1) * R],
                                    rhs=xT_in[c][:], start=(c == 0), stop=(c == nH - 1))
                            u_t = wk.tile([R, NTOK], mdt, tag="u_sb", bufs=2 * T,
                                          name=f"u{nmA}{t3}")
                            nc.vector.tensor_tensor(out=u_t[:], in0=psu[0:R, :],
                                                    in1=masks[t3][:], op=OP.mult)
                            ulist.append(u_t)
                    bq_sb, bv_sb = [], []
                    for nmB, lst in (("bqT", bq_sb), ("bvT", bv_sb)):
                        for t3 in range(T):
                            b_sb = wk.tile([R, H], mdt, tag="b_sb", bufs=2 * T,
                                           name=f"{nmB}{l}_{t3}")
                            nc.sync.dma_start(out=b_sb[:], in_=d[nmB][l, t3])
                            lst.append(b_sb)
                    lora_down = (bq_sb, bv_sb)
                else:
                    lora_down = None

                # ---- Q, K projections (feature-major) ----
                qT, kT = [], []
                for proj, wsb, lst, bias_d, zb in (
                    ("q", wq_sb, qT, "bq", flags["zb_q"]),
                    ("k", wk_sb, kT, "bk", flags["zb_k"]),
                ):
                    for mth in range(nH):
                        ps = pp.tile([P, NTOK], f32, tag="ps512", bufs=2,
                                     name=f"ps{proj}{mth}")
                        for c in range(nH):
                            nc.tensor.matmul(ps[:], lhsT=wsb[c][:, mth * P:(mth + 1) * P],
                                             rhs=xT_in[c][:], start=(c == 0),
                                             stop=(c == nH - 1 and not
                                                   (proj == "q" and lora_down)))
                        if proj == "q" and lora_down is not None:
                            for t3 in range(T):
                                nc.tensor.matmul(
                                    ps[:], lhsT=lora_down[0][t3][:, mth * P:(mth + 1) * P],
                                    rhs=uq[t3][:], start=False, stop=(t3 == T - 1))
                        ot = wk.tile([P, NTOK], mdt, tag="fm512", bufs=2 * nH + 1,
                                     name=f"{proj}T{mth}")
                        if zb:
                            nc.scalar.copy(ot[:], ps[:])
                        else:
                            bt = bias_col(d[bias_d][l, mth * P:(mth + 1) * P, None],
                                          "bcol")
                            nc.scalar.activation(ot[:], ps[:], AF.Identity,
                                                 bias=bt[:, :1])
                        lst.append(ot)

                # ---- V projection (token-major) ----
                v_tiles = []
                for t in range(NT):
                    psv = pp.tile([P, H], f32, tag="ptok", bufs=2, name=f"psv{t}")
                    for (n0, n1) in nsplits:
                        for c in range(nH):
                            nc.tensor.matmul(
                                psv[:, n0:n1], lhsT=xT_in[c][:, t * S:(t + 1) * S],
                                rhs=wv_sb[c][:, n0:n1], start=(c == 0),
                                stop=(c == nH - 1 and lora_down is None))
                        if lora_down is not None:
                            for t3 in range(T):
                                nc.tensor.matmul(
                                    psv[:, n0:n1],
                                    lhsT=uv[t3][:, t * S:(t + 1) * S],
                                    rhs=lora_down[1][t3][:, n0:n1],
                                    start=False, stop=(t3 == T - 1))
                    vt = wk.tile([P, H], mdt, tag="v_sb", bufs=NT + 1, name=f"v{t}")
                    nc.scalar.copy(vt[:], psv[:])
                    v_tiles.append(vt)

                # ---- attention ----
                ctxT = []
                for gidx in range(nH):
                    psc = pp.tile([P, NTOK], f32, tag="ps512", bufs=2, name=f"psc{gidx}")
                    for s in range(NT):
                        for hh in range(P // DH):
                            h = gidx * (P // DH) + hh
                            pss = pp.tile([P, S], f32, tag="pattn", bufs=2,
                                          name=f"pss{s}{hh}")
                            nc.tensor.matmul(
                                pss[:],
                                lhsT=qT[gidx][hh * DH:(hh + 1) * DH, s * S:(s + 1) * S],
                                rhs=kT[gidx][hh * DH:(hh + 1) * DH, s * S:(s + 1) * S],
                                start=True, stop=True)
                            if bias_bc is not None:
                                nc.vector.tensor_tensor(out=pss[:], in0=pss[:],
                                                        in1=bias_bc[s][:], op=OP.add)
                            p_t = wk.tile([P, S], mdt, tag="pexp", bufs=3,
                                          name=f"p{s}{hh}")
                            srow = sm("sm_sum")
                            nc.scalar.activation(p_t[:], pss[:], AF.Exp,
                                                 scale=inv_sqrt_dh, accum_out=srow[:])
                            rec = sm("sm_rec")
                            nc.vector.reciprocal(rec[:], srow[:])
                            nc.vector.tensor_scalar_mul(p_t[:], p_t[:], rec[:, :1])
                            pst = pp.tile([P, S], mdt, tag="pattn", bufs=2,
                                          name=f"pst{s}{hh}")
                            nc.tensor.transpose(pst[:], p_t[:], ident_m[:])
                            pT = wk.tile([P, S], mdt, tag="pT", bufs=3,
                                         name=f"pT{s}{hh}")
                            nc.vector.tensor_copy(pT[:], pst[:])
                            nc.tensor.matmul(
                                psc[hh * DH:(hh + 1) * DH, s * S:(s + 1) * S],
                                lhsT=v_tiles[s][:, h * DH:(h + 1) * DH],
                                rhs=pT[:], start=True, stop=True)
                    ct = wk.tile([P, NTOK], mdt, tag="fm512", bufs=2 * nH + 1,
                                 name=f"ctxT{gidx}")
                    if flags["zb_v"]:
                        nc.scalar.copy(ct[:], psc[:])
                    else:
                        bt = bias_col(d["bv"][l, gidx * P:(gidx + 1) * P, None], "bcol")
                        nc.scalar.activation(ct[:], psc[:], AF.Identity, bias=bt[:, :1])
                    ctxT.append(ct)

                # ---- O projection + residual + LN1 ----
                x1 = []
                for t in range(NT):
                    pso = pp.tile([P, H], f32, tag="ptok", bufs=2, name=f"pso{t}")
                    for (n0, n1) in nsplits:
                        for c in range(nH):
                            nc.tensor.matmul(
                                pso[:, n0:n1], lhsT=ctxT[c][:, t * S:(t + 1) * S],
                                rhs=wo_sb[c][:, n0:n1], start=(c == 0), stop=False)
                        nc.tensor.matmul(pso[:, n0:n1], lhsT=ident_f[:],
                                         rhs=x_in[t][:, n0:n1], start=False, stop=True)
                    if bo_b is not None:
                        nc.vector.tensor_tensor(out=pso[:], in0=pso[:],
                                                in1=bo_b[l][:], op=OP.add)
                    xt = wk.tile([P, H], f32, tag="x1", bufs=NT + 1, name=f"x1_{t}")
                    layernorm(pso[:], xt[:],
                              ln_gb["ln1"][l] if not flags["id_ln1"] else None)
                    x1.append(xt)

                x1T = transpose_to_fm(x1, "xT")

                # ---- FFN1: W1 streamed in column groups of <=6 output chunks ----
                GSZ = min(6, nF)
                assert nF % GSZ == 0
                hT = []
                for grp in range(nF // GSZ):
                    w1g = []
                    for c in range(nH):
                        wt = wk.tile([P, GSZ * P], mdt, tag="w1_sb", bufs=2 * nH,
                                     name=f"w1{l}_{grp}_{c}")
                        nc.sync.dma_start(
                            out=wt[:],
                            in_=d["w1"][l, c * P:(c + 1) * P,
                                        grp * GSZ * P:(grp + 1) * GSZ * P])
                        w1g.append(wt)
                    for ml_ in range(GSZ):
                        mth = grp * GSZ + ml_
                        ps = pp.tile([P, NTOK], f32, tag="ps512", bufs=2,
                                     name=f"psh{mth}")
                        for c in range(nH):
                            nc.tensor.matmul(ps[:],
                                             lhsT=w1g[c][:, ml_ * P:(ml_ + 1) * P],
                                             rhs=x1T[c][:], start=(c == 0),
                                             stop=(c == nH - 1))
                        ht = wk.tile([P, NTOK], mdt, tag="h_sb", bufs=nF + 2,
                                     name=f"hT{mth}")
                        if flags["zb_1"]:
                            nc.scalar.activation(ht[:], ps[:], AF.Gelu_apprx_tanh)
                        else:
                            bt = bias_col(d["b1"][l, mth * P:(mth + 1) * P, None],
                                          "bcol")
                            nc.scalar.activation(ht[:], ps[:], AF.Gelu_apprx_tanh,
                                                 bias=bt[:, :1])
                        hT.append(ht)

                # ---- FFN2: token-tile pairs; W2 restreamed per pair ----
                x_out = [None] * NT
                for pair in [range(i, min(i + 2, NT)) for i in range(0, NT, 2)]:
                    psfs = {}
                    for t in pair:
                        psfs[t] = pp.tile([P, H], f32, tag="ptok", bufs=2,
                                          name=f"psf{t}")
                    for c in range(nF):
                        w2t = wk.tile([P, H], mdt, tag="w2_sb", bufs=6,
                                      name=f"w2{l}_{pair[0]}_{c}")
                        nc.sync.dma_start(out=w2t[:],
                                          in_=d["w2"][l, c * P:(c + 1) * P, :])
                        for t in pair:
                            for (n0, n1) in nsplits:
                                nc.tensor.matmul(
                                    psfs[t][:, n0:n1], lhsT=hT[c][:, t * S:(t + 1) * S],
                                    rhs=w2t[:, n0:n1], start=(c == 0), stop=False)
                    for t in pair:
                        for (n0, n1) in nsplits:
                            nc.tensor.matmul(psfs[t][:, n0:n1], lhsT=ident_f[:],
                                             rhs=x1[t][:, n0:n1], start=False,
                                             stop=True)
                        if b2_b is not None:
                            nc.vector.tensor_tensor(out=psfs[t][:], in0=psfs[t][:],
                                                    in1=b2_b[l][:], op=OP.add)
                        xt = wk.tile([P, H], f32, tag="xw", bufs=NT + 1,
                                     name=f"xo{l}_{t}")
                        layernorm(psfs[t][:], xt[:],
                                  ln_gb["ln2"][l] if not flags["id_ln2"] else None)
                        x_out[t] = xt
                return x_out

            def encoder_pass(lora_masks):
                x = embed()
                xT = transpose_to_fm(x, "xT")
                for l in range(L):
                    x = layer(l, x, xT, lora_masks)
                    if l + 1 < L:
                        xT = transpose_to_fm(x, "xT")
                return x

            def cls_transpose(x_fin, tag):
                """extract CLS rows -> cls [NT, H] fp32; return nH x [P, NT] mdt."""
                cls_t = wk.tile([NT, H], f32, tag=f"cls_{tag}", bufs=1,
                                name=f"cls_{tag}")
                for s in range(NT):
                    nc.sync.dma_start(out=cls_t[s:s + 1, :], in_=x_fin[s][0:1, :])
                outs = []
                for c in range(nH):
                    ps = pp.tile([P, S], f32, tag="pattn", bufs=2, name=f"pscls{c}")
                    nc.tensor.transpose(ps[:, 0:NT], cls_t[0:NT, c * P:(c + 1) * P],
                                        ident_f[0:NT, 0:NT])
                    ct = wk.tile([P, NT], mdt, tag=f"clsT_{tag}", bufs=nH,
                                 name=f"clsT_{tag}{c}")
                    nc.scalar.copy(ct[:], ps[:, 0:NT])
                    outs.append(ct)
                return outs

            # ======================= PASS 1 =======================
            x_fin1 = encoder_pass(None)
            cls1T = cls_transpose(x_fin1, "p1")

            # router MLP
            rfc1_sb = []
            for c in range(nH):
                wt = wk.tile([P, H], mdt, tag="w_enc", bufs=nH + 4, name=f"rf1_{c}")
                nc.sync.dma_start(out=wt[:], in_=d["rfc1"][c * P:(c + 1) * P, :])
                rfc1_sb.append(wt)
            rfc2_sb = wk.tile([P, nH * T], mdt, tag="rf2", bufs=1, name="rf2")
            nc.sync.dma_start(out=rfc2_sb[:].rearrange("p (c t) -> p c t", t=T),
                              in_=d["rfc2"].rearrange("(c p) t -> p c t", p=P))
            z1T = []
            for mth in range(nH):
                ps = pp.tile([P, NTOK], f32, tag="ps512", bufs=2, name=f"psz{mth}")
                for c in range(nH):
                    nc.tensor.matmul(ps[:, 0:NT],
                                     lhsT=rfc1_sb[c][:, mth * P:(mth + 1) * P],
                                     rhs=cls1T[c][:], start=(c == 0),
                                     stop=(c == nH - 1))
                zt = wk.tile([P, NT], mdt, tag="z1T", bufs=nH, name=f"z1T{mth}")
                if flags["zb_r1"]:
                    nc.scalar.activation(zt[:], ps[:, 0:NT], AF.Relu)
                else:
                    bt = bias_col(d["rfc1_b"][mth * P:(mth + 1) * P, None], "bcol")
                    nc.scalar.activation(zt[:], ps[:, 0:NT], AF.Relu, bias=bt[:, :1])
                z1T.append(zt)

            psl = pp.tile([P, S], f32, tag="pattn", bufs=2, name="pslog")
            for c in range(nH):
                nc.tensor.matmul(psl[0:NT, 0:T], lhsT=z1T[c][:, 0:NT],
                                 rhs=rfc2_sb[:, c * T:(c + 1) * T],
                                 start=(c == 0), stop=(c == nH - 1))
            logits = wk.tile([NT, T], f32, tag="logits", bufs=1, name="logits")
            nc.vector.tensor_copy(logits[:], psl[0:NT, 0:T])
            if not flags["zb_r2"]:
                r2b = cpool.tile([NT, T], f32, name="r2b")
                nc.sync.dma_start(out=r2b[:],
                                  in_=d["rfc2_b"][None, :].to_broadcast([NT, T]))
                nc.vector.tensor_tensor(out=logits[:], in0=logits[:], in1=r2b[:],
                                        op=OP.add)
            nc.sync.dma_start(out=out_router[:, :], in_=logits[:])

            # one-hot of argmax + per-task token masks
            rmax = wk.tile([NT, 1], f32, tag="rmax", bufs=1, name="rmax")
            nc.vector.reduce_max(out=rmax[:], in_=logits[:], axis=X)
            onehot = wk.tile([NT, T], f32, tag="onehot", bufs=1, name="onehot")
            nc.vector.tensor_scalar(out=onehot[:], in0=logits[:],
                                    scalar1=rmax[:, :1], scalar2=None,
                                    op0=OP.is_ge)
            masks = []
            for t3 in range(T):
                psm = pp.tile([P, NTOK], f32, tag="ps512", bufs=2, name=f"psm{t3}")
                nc.tensor.matmul(psm[0:R, :],
                                 lhsT=onehot[:, t3:t3 + 1].to_broadcast([NT, R]),
                                 rhs=ssel[:], start=True, stop=True)
                mt = wk.tile([R, NTOK], f32, tag="masks", bufs=T, name=f"mask{t3}")
                nc.vector.tensor_copy(mt[:], psm[0:R, :])
                masks.append(mt)

            # ======================= PASS 2 =======================
            x_fin2 = encoder_pass(masks)
            cls2T = cls_transpose(x_fin2, "p2")

            # per-task heads, masked-combined
            blog = wk.tile([NT, 2], f32, tag="blog", bufs=1, name="blog")
            for t3 in range(T):
                wd_sb = []
                for c in range(nH):
                    wt = wk.tile([P, H], mdt, tag="w_enc", bufs=nH + 4,
                                 name=f"wd{t3}_{c}")
                    nc.sync.dma_start(out=wt[:], in_=d["wd"][t3, c * P:(c + 1) * P, :])
                    wd_sb.append(wt)
                wc_sb = wk.tile([P, nH * 2], mdt, tag="wc_sb", bufs=T, name=f"wc{t3}")
                nc.sync.dma_start(out=wc_sb[:].rearrange("p (c o) -> p c o", o=2),
                                  in_=d["wc"][t3].rearrange("(c p) o -> p c o", p=P))
                dT = []
                for mth in range(nH):
                    ps = pp.tile([P, NTOK], f32, tag="ps512", bufs=2,
                                 name=f"psd{t3}{mth}")
                    for c in range(nH):
                        nc.tensor.matmul(ps[:, 0:NT],
                                         lhsT=wd_sb[c][:, mth * P:(mth + 1) * P],
                                         rhs=cls2T[c][:], start=(c == 0),
                                         stop=(c == nH - 1))
                    dt_ = wk.tile([P, NT], mdt, tag="dT", bufs=nH, name=f"dT{t3}{mth}")
                    if flags["zb_d"]:
                        nc.scalar.activation(dt_[:], ps[:, 0:NT], AF.Tanh)
                    else:
                        bt = bias_col(d["bd"][t3, mth * P:(mth + 1) * P, None], "bcol")
                        nc.scalar.activation(dt_[:], ps[:, 0:NT], AF.Tanh,
                                             bias=bt[:, :1])
                    dT.append(dt_)
                psh = pp.tile([P, S], f32, tag="pattn", bufs=2, name=f"psht{t3}")
                for c in range(nH):
                    nc.tensor.matmul(psh[0:NT, 0:2], lhsT=dT[c][:, 0:NT],
                                     rhs=wc_sb[:, c * 2:(c + 1) * 2],
                                     start=(c == 0), stop=(c == nH - 1))
                lt = wk.tile([NT, 2], f32, tag="lt", bufs=2, name=f"lt{t3}")
                nc.vector.tensor_copy(lt[:], psh[0:NT, 0:2])
                if not flags["zb_c"]:
                    bcb = cpool.tile([NT, 2], f32, name=f"bcb{t3}")
                    nc.sync.dma_start(out=bcb[:],
                                      in_=d["bc"][t3, None, :].to_broadcast([NT, 2]))
                    nc.vector.tensor_tensor(out=lt[:], in0=lt[:], in1=bcb[:], op=OP.add)
                # mask by onehot[:, t3] and accumulate
                nc.vector.tensor_scalar(out=lt[:], in0=lt[:],
                                        scalar1=onehot[:, t3:t3 + 1], scalar2=None,
                                        op0=OP.mult)
                if t3 == 0:
                    nc.vector.tensor_copy(blog[:], lt[:])
                else:
                    nc.vector.tensor_tensor(out=blog[:], in0=blog[:], in1=lt[:],
                                            op=OP.add)
            nc.sync.dma_start(out=out_batch[:, :], in_=blog[:])

    return nc


# ----------------------------------------------------------------------------
# Public entry point
# ----------------------------------------------------------------------------
_CACHE = {}


def _in_maps_for(cfg, host, flags, inputs):
    b16 = ml_dtypes.bfloat16
    ids = np.asarray(inputs["input_ids"]).astype(np.int32)
    base = {
        "word_emb": host["word_emb"], "pos_emb": host["pos_emb"],
        "wq": host["wq"], "wk": host["wk"], "wv": host["wv"], "wo": host["wo"],
        "w1": host["w1"], "w2": host["w2"],
        "aqT": host["aqT"], "avT": host["avT"],
        "bqT": host["bqT"], "bvT": host["bvT"],
        "rfc1": host["rfc1"], "rfc2": host["rfc2"],
        "wd": host["wd"], "wc": host["wc"],
    }
    if not flags["zb_q"]: base["bq"] = host["bq"]
    if not flags["zb_k"]: base["bk"] = host["bk"]
    if not flags["zb_v"]: base["bv"] = host["bv"]
    if not flags["zb_o"]: base["bo"] = host["bo"]
    if not flags["zb_1"]: base["b1"] = host["b1"]
    if not flags["zb_2"]: base["b2"] = host["b2"]
    if not flags["zb_r1"]: base["rfc1_b"] = host["rfc1_b"]
    if not flags["zb_r2"]: base["rfc2_b"] = host["rfc2_b"]
    if not flags["zb_d"]: base["bd"] = host["bd"]
    if not flags["zb_c"]: base["bc"] = host["bc"]
    if not flags["id_emb_ln"]:
        base["emb_ln_g"] = host["emb_ln_g"]; base["emb_ln_b"] = host["emb_ln_b"]
    if not flags["id_ln1"]:
        base["ln1_g"] = host["ln1_g"]; base["ln1_b"] = host["ln1_b"]
    if not flags["id_ln2"]:
        base["ln2_g"] = host["ln2_g"]; base["ln2_b"] = host["ln2_b"]

    NT = cfg["B"] // cfg["n_cores"]
    in_maps = []
    for c in range(cfg["n_cores"]):
        m = dict(base)
        m["ids"] = np.ascontiguousarray(ids[c * NT:(c + 1) * NT])
        if not flags["mask_ones"]:
            am = np.asarray(inputs["attention_mask"]).astype(np.float32)
            bias = (am - 1.0) * np.float32(1e9 / np.sqrt(cfg["DH"]))
            m["attn_bias"] = np.ascontiguousarray(bias[c * NT:(c + 1) * NT])
        in_maps.append(m)
    return in_maps


def _ensure_axon_hooks():
    """Provide antenv.axon_hooks (missing in this image) so trace=True can
    capture NTFF profiles through the axon tunnel.  Mirrors
    trn_agent_boot.trn_boot._ntff_profile_via_ctypes."""
    import sys, types, ctypes, contextlib

    try:
        from antenv.axon_hooks import get_axon_ntff_profile_hook  # noqa
        return
    except ImportError:
        pass

    so_path = "/opt/axon/libaxon_pjrt.so"
    import os
    mod = types.ModuleType("antenv.axon_hooks")
    state = {"hook": None}

    def set_axon_ntff_profile_hook(h):
        state["hook"] = h

    def get_axon_ntff_profile_hook():
        return state["hook"]

    mod.set_axon_ntff_profile_hook = set_axon_ntff_profile_hook
    mod.get_axon_ntff_profile_hook = get_axon_ntff_profile_hook
    sys.modules["antenv.axon_hooks"] = mod
    import antenv
    antenv.axon_hooks = mod

    if not os.path.exists(so_path):
        return
    lib = ctypes.CDLL(so_path)
    if not hasattr(lib, "axon_start_nrt_profile"):
        return
    lib.axon_start_nrt_profile.argtypes = [ctypes.POINTER(ctypes.c_int64),
                                           ctypes.c_size_t]
    lib.axon_start_nrt_profile.restype = ctypes.c_int64
    lib.axon_stop_nrt_profile.argtypes = [ctypes.c_char_p]
    lib.axon_stop_nrt_profile.restype = ctypes.c_int64

    @contextlib.contextmanager
    def _hook(output_dir, device_ids):
        import jax
        jax.devices()
        if device_ids:
            ids = (ctypes.c_int64 * len(device_ids))(*device_ids)
            rc = lib.axon_start_nrt_profile(ids, len(device_ids))
        else:
            rc = lib.axon_start_nrt_profile(None, 0)
        if rc != 0:
            raise RuntimeError(f"axon_start_nrt_profile rc={rc}")
        try:
            yield
        finally:
            n = lib.axon_stop_nrt_profile(str(output_dir).encode())
            print(f"ntff profile: {n} file(s) written to {output_dir}",
                  file=sys.stderr)

    set_axon_ntff_profile_hook(_hook)


def run(inputs, cfg=None, mm_dt_name="bfloat16", trace=False):
    from concourse.bass_utils import run_bass_kernel_spmd
    if trace:
        _ensure_axon_hooks()

    cfg = dict(REAL_CFG if cfg is None else cfg)
    mm_np = ml_dtypes.bfloat16 if mm_dt_name == "bfloat16" else np.float32
    host, flags = _prep_host(inputs, cfg, mm_np)

    key = (tuple(sorted(cfg.items())), tuple(sorted(flags.items())), mm_dt_name)
    if key not in _CACHE:
        _CACHE[key] = _build(cfg, flags, mm_dt_name)
    nc = _CACHE[key]

    in_maps = _in_maps_for(cfg, host, flags, inputs)
    res = run_bass_kernel_spmd(nc, in_maps, core_ids=list(range(cfg["n_cores"])),
                               trace=trace)
    NT = cfg["B"] // cfg["n_cores"]
    rl = np.concatenate([r["router_logits"] for r in res.results], axis=0)
    bl = np.concatenate([r["batch_logits"] for r in res.results], axis=0)
    return (rl.astype(np.float32), bl.astype(np.float32)), res


def kernel(**inputs):
    (rl, bl), _ = run(inputs)
    return rl, bl


# revision 15
# speedup vs baseline: 1.0585x; 1.0585x over previous
"""Trainium2 Bass kernel for nn_EndToEndModel_86552180949694 (moe_routing).

RoBERTa-like 4-layer encoder run twice per sample batch:
  pass 1 (no LoRA) -> CLS -> router MLP -> argmax task id
  pass 2 (per-sample LoRA on q/v) -> CLS -> per-task classification head
Returns (router_logits [B,3], batch_logits [B,2]).

Sharding: pure data-parallel over batch, 4 samples per core on 8 cores.
All weights replicated; zero cross-core communication.  MoE dispatch is
realized densely: LoRA / head computed for all T=3 tasks and combined with
an argmax one-hot mask (rank-16 adapters make this ~8% extra FLOPs).

Layouts per core (P=128, S=128 tokens/sample, 4 samples = 4 "token tiles"):
  token-major  [128 tokens, H]  - residual stream, LN over free dim
  feature-major [H rows, 512 tokens] ("xT") - matmul operand layout
Matmul convention: out[M,N] = lhsT[K,M].T @ rhs[K,N], K = partitions.
"""

import numpy as np
import ml_dtypes

# ----------------------------------------------------------------------------
# Real model dimensions (hardcoded; kernel.py must be self-contained)
# ----------------------------------------------------------------------------
REAL_CFG = dict(
    B=32, S=128, V=50265, H=768, L=4, NH=12, DH=64, FF=3072, T=3, R=16,
    n_cores=8, lora_scale=2.0, eps=1e-5,
)

P = 128


def _install_tilefix():
    """Walrus in this container only allows 1 sync-wait on an SP Drain;
    Tile's tail drain attaches one wait per DMA queue.  Split them across
    several single-wait Drain instructions."""
    import concourse.mybir as mybir
    import concourse.tile as tile_mod
    from concourse.vector_clock import ScopedClock

    if getattr(tile_mod.TileContext, "_drain_split_installed", False):
        return

    def _drain_and_barrier(self, tick_clock, wait_clock):
        nc = self.nc
        drain_inst = nc.sync.drain()
        wait_clock.add_sem_waits(
            drain_inst.ins, ScopedClock({None: tick_clock.global_clock})
        )
        si = drain_inst.ins.sync_info
        if si is not None and si.on_wait and len(si.on_wait) > 1:
            waits = list(si.on_wait)
            drain_inst.ins.sync_info = mybir.SyncInfo(
                on_wait=[waits[0]], on_update=list(si.on_update)
            )
            for w in waits[1:]:
                d2 = nc.sync.drain()
                d2.ins.sync_info = mybir.SyncInfo(on_wait=[w], on_update=[])
        nc.all_engine_barrier()
        assert self.sems is not None
        popped = nc._tile_sem_poison_stack.pop()
        assert popped is self._sem_poison
        nc.clear_and_free_semaphores(list(self.sems.allocated().values()))
        nc.all_engine_barrier()

    tile_mod.TileContext._drain_and_barrier = _drain_and_barrier

    # Generalized: ANY instruction with >1 sync wait gets its extra waits
    # peeled onto single-wait NoOp carriers inserted just before it (same
    # engine, so sequential semantics are preserved).
    import bass_rust
    _orig_add = tile_mod.TileContext._add_instruction

    def _add_instruction_split(self, inst):
        si = getattr(inst, "sync_info", None)
        if si is not None and si.on_wait is not None and len(si.on_wait) > 1:
            waits = list(si.on_wait)
            eng = inst.engine
            if eng is not None and eng != mybir.EngineType.Unassigned:
                for w in waits[:-1]:
                    nop = bass_rust.InstNoOp(
                        name=f"waitnop-{self.nc.next_id()}", ins=[], outs=[])
                    nop.engine = eng
                    nop.sync_info = mybir.SyncInfo(on_wait=[w], on_update=[])
                    _orig_add(self, nop)
                inst.sync_info = mybir.SyncInfo(
                    on_wait=[waits[-1]], on_update=list(si.on_update))
        _orig_add(self, inst)

    tile_mod.TileContext._add_instruction = _add_instruction_split
    tile_mod.TileContext._drain_split_installed = True


# ----------------------------------------------------------------------------
# Host-side preprocessing
# ----------------------------------------------------------------------------
def _prep_host(inputs, cfg, mm_np):
    """Cast/transform weights, detect trivial biases / identity LN."""
    f32 = np.float32
    H, L, T, R = cfg["H"], cfg["L"], cfg["T"], cfg["R"]

    def az(x):  # all-zero
        return bool(np.all(np.asarray(x) == 0))

    def ao(x):  # all-one
        return bool(np.all(np.asarray(x) == 1))

    g = {k: np.asarray(v) for k, v in inputs.items()}

    host = {}
    host["word_emb"] = g["word_emb"].astype(f32)
    host["pos_emb"] = g["pos_emb"].astype(f32)
    for nm in ("Wq", "Wk", "Wv", "Wo"):
        host[nm.lower()] = np.ascontiguousarray(g[nm].astype(mm_np))
    host["w1"] = np.ascontiguousarray(g["W1"].astype(mm_np))
    host["w2"] = np.ascontiguousarray(g["W2"].astype(mm_np))
    # LoRA: A [T,L,R,H] -> aT [L,T,H,R]; B [T,L,H,R] -> bT [L,T,R,H] (x scale)
    sc = cfg["lora_scale"]
    host["aqT"] = np.ascontiguousarray(np.transpose(g["Aq"], (1, 0, 3, 2)).astype(mm_np))
    host["avT"] = np.ascontiguousarray(np.transpose(g["Av"], (1, 0, 3, 2)).astype(mm_np))
    host["bqT"] = np.ascontiguousarray((sc * np.transpose(g["Bq"], (1, 0, 3, 2))).astype(mm_np))
    host["bvT"] = np.ascontiguousarray((sc * np.transpose(g["Bv"], (1, 0, 3, 2))).astype(mm_np))
    # router (fold 1/max(temp, .05) into fc2)
    scale_r = 1.0 / max(float(np.asarray(g["r_temp"]).reshape(-1)[0]), 0.05)
    host["rfc1"] = np.ascontiguousarray(g["r_fc1_W"].astype(mm_np))
    host["rfc2"] = np.ascontiguousarray((g["r_fc2_W"] * scale_r).astype(mm_np))
    host["rfc1_b"] = g["r_fc1_b"].astype(f32)
    host["rfc2_b"] = (g["r_fc2_b"] * scale_r).astype(f32)
    # heads
    host["wd"] = np.ascontiguousarray(g["cls_dense_W"].astype(mm_np))
    host["wc"] = np.ascontiguousarray(g["cls_out_W"].astype(mm_np))
    host["bd"] = g["cls_dense_b"].astype(f32)
    host["bc"] = g["cls_out_b"].astype(f32)
    # biases / LN params (fp32)
    for nm in ("bq", "bk", "bv", "bo", "b1", "b2"):
        host[nm] = g[nm].astype(f32)
    for nm in ("emb_ln_g", "emb_ln_b", "ln1_g", "ln1_b", "ln2_g", "ln2_b"):
        host[nm] = g[nm].astype(f32)

    flags = dict(
        zb_q=az(g["bq"]), zb_k=az(g["bk"]), zb_v=az(g["bv"]), zb_o=az(g["bo"]),
        zb_1=az(g["b1"]), zb_2=az(g["b2"]),
        zb_r1=az(g["r_fc1_b"]), zb_r2=az(host["rfc2_b"]),
        zb_d=az(g["cls_dense_b"]), zb_c=az(g["cls_out_b"]),
        id_emb_ln=ao(g["emb_ln_g"]) and az(g["emb_ln_b"]),
        id_ln1=ao(g["ln1_g"]) and az(g["ln1_b"]),
        id_ln2=ao(g["ln2_g"]) and az(g["ln2_b"]),
        mask_ones=ao(g["attention_mask"]),
    )
    return host, flags


# ----------------------------------------------------------------------------
# Device program builder
# ----------------------------------------------------------------------------
def _build(cfg, flags, mm_dt_name="bfloat16"):
    import concourse.bass as bass
    import concourse.mybir as mybir
    from concourse.tile import TileContext
    from concourse.masks import make_identity

    _install_tilefix()

    f32 = mybir.dt.float32
    i32 = mybir.dt.int32
    mdt = getattr(mybir.dt, mm_dt_name)
    AF = mybir.ActivationFunctionType
    OP = mybir.AluOpType
    X = mybir.AxisListType.X

    S, H, L, NH, DH, FF, T, R, V = (cfg[k] for k in
                                    ("S", "H", "L", "NH", "DH", "FF", "T", "R", "V"))
    NT = cfg["B"] // cfg["n_cores"]
    NTOK = NT * S
    nH, nF = H // P, FF // P
    assert S == P and DH == 64 and H % P == 0 and FF % P == 0
    eps = cfg["eps"]
    inv_sqrt_dh = 1.0 / float(np.sqrt(DH))

    nc = bass.Bass("TRN2", target_bir_lowering=False, debug=False,
                   num_devices=cfg["n_cores"])

    d = {}
    def din(name, shape, dt):
        d[name] = nc.dram_tensor(name, list(shape), dt, kind="ExternalInput")
        return d[name]

    din("ids", (NT, S), i32)
    din("word_emb", (V, H), f32)
    din("pos_emb", (S, H), f32)
    for nm in ("wq", "wk", "wv", "wo"):
        din(nm, (L, H, H), mdt)
    din("w1", (L, H, FF), mdt)
    din("w2", (L, FF, H), mdt)
    din("aqT", (L, T, H, R), mdt)
    din("avT", (L, T, H, R), mdt)
    din("bqT", (L, T, R, H), mdt)
    din("bvT", (L, T, R, H), mdt)
    din("rfc1", (H, H), mdt)
    din("rfc2", (H, T), mdt)
    din("wd", (T, H, H), mdt)
    din("wc", (T, H, 2), mdt)
    if not flags["zb_q"]: din("bq", (L, H), f32)
    if not flags["zb_k"]: din("bk", (L, H), f32)
    if not flags["zb_v"]: din("bv", (L, H), f32)
    if not flags["zb_o"]: din("bo", (L, H), f32)
    if not flags["zb_1"]: din("b1", (L, FF), f32)
    if not flags["zb_2"]: din("b2", (L, H), f32)
    if not flags["zb_r1"]: din("rfc1_b", (H,), f32)
    if not flags["zb_r2"]: din("rfc2_b", (T,), f32)
    if not flags["zb_d"]: din("bd", (T, H), f32)
    if not flags["zb_c"]: din("bc", (T, 2), f32)
    if not flags["id_emb_ln"]:
        din("emb_ln_g", (H,), f32); din("emb_ln_b", (H,), f32)
    if not flags["id_ln1"]:
        din("ln1_g", (L, H), f32); din("ln1_b", (L, H), f32)
    if not flags["id_ln2"]:
        din("ln2_g", (L, H), f32); din("ln2_b", (L, H), f32)
    if not flags["mask_ones"]:
        din("attn_bias", (NT, S), f32)

    out_router = nc.dram_tensor("router_logits", [NT, T], f32, kind="ExternalOutput")
    out_batch = nc.dram_tensor("batch_logits", [NT, 2], f32, kind="ExternalOutput")

    # N-split for token-major matmul outputs of width H (fp32 PSUM bank = 512)
    nsplits = [(i, min(i + 512, H)) for i in range(0, H, 512)]

    with TileContext(nc) as tc:
        import contextlib
        ctx = contextlib.ExitStack()
        with ctx:
            cpool = ctx.enter_context(tc.tile_pool(name="const", bufs=1))
            wk = ctx.enter_context(tc.tile_pool(name="work", bufs=1))
            pp = ctx.enter_context(tc.tile_pool(name="ps", bufs=1, space="PSUM"))

            # ---------------- constants ----------------
            ident_f = cpool.tile([P, P], f32, name="ident_f")
            make_identity(nc, ident_f[:])
            ident_m = cpool.tile([P, P], mdt, name="ident_m")
            make_identity(nc, ident_m[:])
            pos_t = cpool.tile([P, H], f32, name="pos_t")
            nc.sync.dma_start(out=pos_t[:], in_=d["pos_emb"][:, :])
            eps_t = cpool.tile([P, 1], f32, name="eps_t")
            nc.gpsimd.memset(eps_t[:], eps)
            ssel = cpool.tile([NT, NTOK], f32, name="ssel")
            nc.gpsimd.memset(ssel[:], 0.0)
            nc.gpsimd.affine_select(
                out=ssel[:], in_=ssel[:], compare_op=OP.not_equal, fill=1.0,
                base=0, pattern=[[-1, NT], [0, S]], channel_multiplier=1,
            )

            # LN gamma/beta broadcast tiles (only when non-identity)
            ln_gb = {}
            def bcast_row(name, src_ap, width=None):
                w = H if width is None else width
                t_b = cpool.tile([P, w], f32, name=name + "_b")
                nc.sync.dma_start(out=t_b[:], in_=src_ap.to_broadcast([P, w]))
                return t_b
            if not flags["id_emb_ln"]:
                ln_gb["emb"] = (bcast_row("embg", d["emb_ln_g"][None, :]),
                                bcast_row("embb", d["emb_ln_b"][None, :]))
            if not flags["id_ln1"]:
                ln_gb["ln1"] = [(bcast_row(f"l1g{l}", d["ln1_g"][l, None, :]),
                                 bcast_row(f"l1b{l}", d["ln1_b"][l, None, :]))
                                for l in range(L)]
            if not flags["id_ln2"]:
                ln_gb["ln2"] = [(bcast_row(f"l2g{l}", d["ln2_g"][l, None, :]),
                                 bcast_row(f"l2b{l}", d["ln2_b"][l, None, :]))
                                for l in range(L)]
            b2_b = bo_b = None
            if not flags["zb_2"]:
                b2_b = [bcast_row(f"b2_{l}", d["b2"][l, None, :]) for l in range(L)]
            if not flags["zb_o"]:
                bo_b = [bcast_row(f"bo_{l}", d["bo"][l, None, :]) for l in range(L)]

            # attention-mask additive bias (rare path)
            bias_bc = None
            if not flags["mask_ones"]:
                bias_bc = []
                for s in range(NT):
                    mb = cpool.tile([P, S], f32, name=f"mb{s}")
                    nc.sync.dma_start(
                        out=mb[:],
                        in_=d["attn_bias"][s, None, :].to_broadcast([P, S]))
                    bias_bc.append(mb)

            # ---------------- small helpers ----------------
            def sm(tag):
                return wk.tile([P, 1], f32, tag=tag, bufs=6, name=tag)

            def layernorm(y_ap, x_out_ap, gb):
                """token-major LN over free dim H: x_out = (y-m)/sqrt(var+eps)*g+b
                y_ap may be PSUM or SBUF fp32."""
                sums = sm("ln_sum"); sqs = sm("ln_sq")
                nc.vector.reduce_sum(out=sums[:], in_=y_ap, axis=X)
                junk = wk.tile([P, H], f32, tag="scr768", bufs=2, name="ln_junk")
                nc.scalar.activation(junk[:], y_ap, AF.Square, accum_out=sqs[:])
                m = sm("ln_m"); ex2 = sm("ln_e2"); var = sm("ln_v")
                nc.vector.tensor_scalar_mul(m[:], sums[:], 1.0 / H)
                nc.vector.tensor_scalar_mul(ex2[:], sqs[:], 1.0 / H)
                nc.vector.tensor_tensor(out=var[:], in0=m[:], in1=m[:], op=OP.mult)
                nc.vector.tensor_tensor(out=var[:], in0=ex2[:], in1=var[:], op=OP.subtract)
                std = sm("ln_std"); rstd = sm("ln_rs"); negmr = sm("ln_nm")
                nc.scalar.activation(std[:], var[:], AF.Sqrt, bias=eps_t[:, :1])
                nc.vector.reciprocal(rstd[:], std[:])
                # HAM keepalive: ~250ns PE op dependent on the LN chain so the
                # scheduler places it inside the otherwise PE-idle LN window.
                ka = pp.tile([P, S], f32, tag="pattn", bufs=2, name="ln_ka")
                nc.tensor.transpose(ka[0:1, 0:P], rstd[:, 0:1], ident_f[:])
                nc.vector.tensor_scalar(out=negmr[:], in0=m[:], scalar1=rstd[:, :1],
                                        scalar2=-1.0, op0=OP.mult, op1=OP.mult)
                nc.vector.tensor_scalar(out=x_out_ap, in0=y_ap, scalar1=rstd[:, :1],
                                        scalar2=negmr[:, :1], op0=OP.mult, op1=OP.add)
                if gb is not None:
                    gt, bt = gb
                    nc.vector.tensor_tensor(out=x_out_ap, in0=x_out_ap, in1=gt[:], op=OP.mult)
                    nc.vector.tensor_tensor(out=x_out_ap, in0=x_out_ap, in1=bt[:], op=OP.add)

            def transpose_to_fm(x_tiles, tag):
                """token-major fp32 NT tiles [P,H] -> feature-major mdt tiles
                nH x [P, NTOK]."""
                outs = []
                for c in range(nH):
                    ps = pp.tile([P, NTOK], f32, tag="ps512", bufs=2, name=f"pt{c}")
                    for t in range(NT):
                        nc.tensor.transpose(ps[:, t * S:(t + 1) * S],
                                            x_tiles[t][:, c * P:(c + 1) * P],
                                            ident_f[:])
                    xt = wk.tile([P, NTOK], mdt, tag=tag, bufs=nH + 2, name=f"{tag}{c}")
                    nc.scalar.copy(xt[:], ps[:])
                    outs.append(xt)
                return outs

            def bias_col(dram_ap, tag):
                """DMA a [128] bias slice to a [128,1] fp32 SBUF tile."""
                bt = wk.tile([P, 1], f32, tag=tag, bufs=4, name=tag)
                nc.sync.dma_start(out=bt[:], in_=dram_ap)
                return bt

            # ---------------- embedding (run once per pass) ----------------
            def embed():
                x0 = []
                for t in range(NT):
                    ids_t = wk.tile([P, 1], i32, tag="ids", bufs=NT, name=f"ids{t}")
                    nc.sync.dma_start(out=ids_t[:], in_=d["ids"][t, :, None])
                    emb = wk.tile([P, H], f32, tag="scr768", bufs=2, name=f"emb{t}")
                    nc.gpsimd.indirect_dma_start(
                        out=emb[:], out_offset=None, in_=d["word_emb"][:, :],
                        in_offset=bass.IndirectOffsetOnAxis(ap=ids_t[:, :1], axis=0),
                    )
                    nc.vector.tensor_tensor(out=emb[:], in0=emb[:], in1=pos_t[:],
                                            op=OP.add)
                    xt = wk.tile([P, H], f32, tag="xw", bufs=NT + 1, name=f"x0_{t}")
                    layernorm(emb[:], xt[:], ln_gb.get("emb"))
                    x0.append(xt)
                return x0

            # ---------------- encoder layer ----------------
            def layer(l, x_in, xT_in, lora_ctx):
                """One encoder layer.  x_in: NT token-major fp32 tiles.
                xT_in: nH feature-major mdt tiles.  Returns (x_out, xT_out_none)."""
                # ---- weight chunk DMAs ----
                wq_sb = []; wk_sb = []; wv_sb = []; wo_sb = []
                for nm, lst in (("wq", wq_sb), ("wk", wk_sb),
                                ("wv", wv_sb), ("wo", wo_sb)):
                    for c in range(nH):
                        wt = wk.tile([P, H], mdt, tag="w_enc", bufs=nH + 4,
                                     name=f"{nm}{l}_{c}")
                        nc.sync.dma_start(out=wt[:], in_=d[nm][l, c * P:(c + 1) * P, :])
                        lst.append(wt)

                # ---- LoRA u vectors (pass 2) ----
                uq = uv = None
                if lora_ctx is not None:
                    masks = lora_ctx
                    uq, uv = [], []
                    for nmA, ulist in (("aqT", uq), ("avT", uv)):
                        for t3 in range(T):
                            a_sb = wk.tile([P, nH * R], mdt, tag="a_sb", bufs=4,
                                           name=f"{nmA}{l}_{t3}")
                            nc.sync.dma_start(
                                out=a_sb[:].rearrange("p (c r) -> p c r", r=R),
                                in_=d[nmA][l, t3].rearrange("(c p) r -> p c r", p=P))
                            psu = pp.tile([P, NTOK], f32, tag="ps512", bufs=2,
                                          name=f"psu{t3}")
                            for c in range(nH):
                                nc.tensor.matmul(
                                    psu[0:R, :], lhsT=a_sb[:, c * R:(c + 1) * R],
                                    rhs=xT_in[c][:], start=(c == 0), stop=(c == nH - 1))
                            u_t = wk.tile([R, NTOK], mdt, tag="u_sb", bufs=2 * T,
                                          name=f"u{nmA}{t3}")
                            nc.vector.tensor_tensor(out=u_t[:], in0=psu[0:R, :],
                                                    in1=masks[t3][:], op=OP.mult)
                            ulist.append(u_t)
                    bq_sb, bv_sb = [], []
                    for nmB, lst in (("bqT", bq_sb), ("bvT", bv_sb)):
                        for t3 in range(T):
                            b_sb = wk.tile([R, H], mdt, tag="b_sb", bufs=2 * T,
                                           name=f"{nmB}{l}_{t3}")
                            nc.sync.dma_start(out=b_sb[:], in_=d[nmB][l, t3])
                            lst.append(b_sb)
                    lora_down = (bq_sb, bv_sb)
                else:
                    lora_down = None

                # ---- Q, K projections (feature-major) ----
                qT, kT = [], []
                for proj, wsb, lst, bias_d, zb in (
                    ("q", wq_sb, qT, "bq", flags["zb_q"]),
                    ("k", wk_sb, kT, "bk", flags["zb_k"]),
                ):
                    for mth in range(nH):
                        ps = pp.tile([P, NTOK], f32, tag="ps512", bufs=2,
                                     name=f"ps{proj}{mth}")
                        for c in range(nH):
                            nc.tensor.matmul(ps[:], lhsT=wsb[c][:, mth * P:(mth + 1) * P],
                                             rhs=xT_in[c][:], start=(c == 0),
                                             stop=(c == nH - 1 and not
                                                   (proj == "q" and lora_down)))
                        if proj == "q" and lora_down is not None:
                            for t3 in range(T):
                                nc.tensor.matmul(
                                    ps[:], lhsT=lora_down[0][t3][:, mth * P:(mth + 1) * P],
                                    rhs=uq[t3][:], start=False, stop=(t3 == T - 1))
                        ot = wk.tile([P, NTOK], mdt, tag="fm512", bufs=2 * nH + 1,
                                     name=f"{proj}T{mth}")
                        if zb:
                            nc.scalar.copy(ot[:], ps[:])
                        else:
                            bt = bias_col(d[bias_d][l, mth * P:(mth + 1) * P, None],
                                          "bcol")
                            nc.scalar.activation(ot[:], ps[:], AF.Identity,
                                                 bias=bt[:, :1])
                        lst.append(ot)

                # ---- V projection (token-major) ----
                v_tiles = []
                for t in range(NT):
                    psv = pp.tile([P, H], f32, tag="ptok", bufs=2, name=f"psv{t}")
                    for (n0, n1) in nsplits:
                        for c in range(nH):
                            nc.tensor.matmul(
                                psv[:, n0:n1], lhsT=xT_in[c][:, t * S:(t + 1) * S],
                                rhs=wv_sb[c][:, n0:n1], start=(c == 0),
                                stop=(c == nH - 1 and lora_down is None))
                        if lora_down is not None:
                            for t3 in range(T):
                                nc.tensor.matmul(
                                    psv[:, n0:n1],
                                    lhsT=uv[t3][:, t * S:(t + 1) * S],
                                    rhs=lora_down[1][t3][:, n0:n1],
                                    start=False, stop=(t3 == T - 1))
                    vt = wk.tile([P, H], mdt, tag="v_sb", bufs=NT + 1, name=f"v{t}")
                    nc.scalar.copy(vt[:], psv[:])
                    v_tiles.append(vt)

                # ---- attention ----
                ctxT = []
                for gidx in range(nH):
                    psc = pp.tile([P, NTOK], f32, tag="ps512", bufs=2, name=f"psc{gidx}")
                    for s in range(NT):
                        for hh in range(P // DH):
                            h = gidx * (P // DH) + hh
                            alt = (s * (P // DH) + hh) % 2
                            pss = pp.tile([P, S], f32,
                                          tag=("pattn" if alt else "ptok"),
                                          bufs=2, name=f"pss{s}{hh}")
                            nc.tensor.matmul(
                                pss[:],
                                lhsT=qT[gidx][hh * DH:(hh + 1) * DH, s * S:(s + 1) * S],
                                rhs=kT[gidx][hh * DH:(hh + 1) * DH, s * S:(s + 1) * S],
                                start=True, stop=True)
                            if bias_bc is not None:
                                nc.vector.tensor_tensor(out=pss[:], in0=pss[:],
                                                        in1=bias_bc[s][:], op=OP.add)
                            p_t = wk.tile([P, S], mdt, tag="pexp", bufs=6,
                                          name=f"p{s}{hh}")
                            srow = sm("sm_sum")
                            nc.scalar.activation(p_t[:], pss[:], AF.Exp,
                                                 scale=inv_sqrt_dh, accum_out=srow[:])
                            rec = sm("sm_rec")
                            nc.vector.reciprocal(rec[:], srow[:])
                            nc.vector.tensor_scalar_mul(p_t[:], p_t[:], rec[:, :1])
                            pst = pp.tile([P, S], mdt, tag="pattn", bufs=2,
                                          name=f"pst{s}{hh}")
                            nc.tensor.transpose(pst[:], p_t[:], ident_m[:])
                            pT = wk.tile([P, S], mdt, tag="pT", bufs=6,
                                         name=f"pT{s}{hh}")
                            nc.vector.tensor_copy(pT[:], pst[:])
                            nc.tensor.matmul(
                                psc[hh * DH:(hh + 1) * DH, s * S:(s + 1) * S],
                                lhsT=v_tiles[s][:, h * DH:(h + 1) * DH],
                                rhs=pT[:], start=True, stop=True)
                    ct = wk.tile([P, NTOK], mdt, tag="fm512", bufs=2 * nH + 1,
                                 name=f"ctxT{gidx}")
                    if flags["zb_v"]:
                        nc.scalar.copy(ct[:], psc[:])
                    else:
                        bt = bias_col(d["bv"][l, gidx * P:(gidx + 1) * P, None], "bcol")
                        nc.scalar.activation(ct[:], psc[:], AF.Identity, bias=bt[:, :1])
                    ctxT.append(ct)

                # ---- O projection + residual + LN1 ----
                x1 = []
                for t in range(NT):
                    pso = pp.tile([P, H], f32, tag="ptok", bufs=2, name=f"pso{t}")
                    for (n0, n1) in nsplits:
                        for c in range(nH):
                            nc.tensor.matmul(
                                pso[:, n0:n1], lhsT=ctxT[c][:, t * S:(t + 1) * S],
                                rhs=wo_sb[c][:, n0:n1], start=(c == 0), stop=False)
                        nc.tensor.matmul(pso[:, n0:n1], lhsT=ident_f[:],
                                         rhs=x_in[t][:, n0:n1], start=False, stop=True)
                    if bo_b is not None:
                        nc.vector.tensor_tensor(out=pso[:], in0=pso[:],
                                                in1=bo_b[l][:], op=OP.add)
                    xt = wk.tile([P, H], f32, tag="x1", bufs=NT + 1, name=f"x1_{t}")
                    layernorm(pso[:], xt[:],
                              ln_gb["ln1"][l] if not flags["id_ln1"] else None)
                    x1.append(xt)

                x1T = transpose_to_fm(x1, "xT")

                # ---- FFN1: W1 streamed in column groups of <=6 output chunks ----
                GSZ = min(6, nF)
                assert nF % GSZ == 0
                hT = []
                for grp in range(nF // GSZ):
                    w1g = []
                    for c in range(nH):
                        wt = wk.tile([P, GSZ * P], mdt, tag="w1_sb", bufs=2 * nH,
                                     name=f"w1{l}_{grp}_{c}")
                        nc.sync.dma_start(
                            out=wt[:],
                            in_=d["w1"][l, c * P:(c + 1) * P,
                                        grp * GSZ * P:(grp + 1) * GSZ * P])
                        w1g.append(wt)
                    for ml_ in range(GSZ):
                        mth = grp * GSZ + ml_
                        ps = pp.tile([P, NTOK], f32, tag="ps512", bufs=2,
                                     name=f"psh{mth}")
                        for c in range(nH):
                            nc.tensor.matmul(ps[:],
                                             lhsT=w1g[c][:, ml_ * P:(ml_ + 1) * P],
                                             rhs=x1T[c][:], start=(c == 0),
                                             stop=(c == nH - 1))
                        ht = wk.tile([P, NTOK], mdt, tag="h_sb", bufs=nF + 2,
                                     name=f"hT{mth}")
                        if flags["zb_1"]:
                            nc.scalar.activation(ht[:], ps[:], AF.Gelu_apprx_tanh)
                        else:
                            bt = bias_col(d["b1"][l, mth * P:(mth + 1) * P, None],
                                          "bcol")
                            nc.scalar.activation(ht[:], ps[:], AF.Gelu_apprx_tanh,
                                                 bias=bt[:, :1])
                        hT.append(ht)

                # ---- FFN2: token-tile pairs; W2 restreamed per pair ----
                x_out = [None] * NT
                for pair in [range(i, min(i + 2, NT)) for i in range(0, NT, 2)]:
                    psfs = {}
                    for t in pair:
                        psfs[t] = pp.tile([P, H], f32, tag="ptok", bufs=2,
                                          name=f"psf{t}")
                    for c in range(nF):
                        w2t = wk.tile([P, H], mdt, tag="w2_sb", bufs=6,
                                      name=f"w2{l}_{pair[0]}_{c}")
                        nc.sync.dma_start(out=w2t[:],
                                          in_=d["w2"][l, c * P:(c + 1) * P, :])
                        for t in pair:
                            for (n0, n1) in nsplits:
                                nc.tensor.matmul(
                                    psfs[t][:, n0:n1], lhsT=hT[c][:, t * S:(t + 1) * S],
                                    rhs=w2t[:, n0:n1], start=(c == 0), stop=False)
                    for t in pair:
                        for (n0, n1) in nsplits:
                            nc.tensor.matmul(psfs[t][:, n0:n1], lhsT=ident_f[:],
                                             rhs=x1[t][:, n0:n1], start=False,
                                             stop=True)
                        if b2_b is not None:
                            nc.vector.tensor_tensor(out=psfs[t][:], in0=psfs[t][:],
                                                    in1=b2_b[l][:], op=OP.add)
                        xt = wk.tile([P, H], f32, tag="xw", bufs=NT + 1,
                                     name=f"xo{l}_{t}")
                        layernorm(psfs[t][:], xt[:],
                                  ln_gb["ln2"][l] if not flags["id_ln2"] else None)
                        x_out[t] = xt
                return x_out

            def encoder_pass(lora_masks):
                x = embed()
                xT = transpose_to_fm(x, "xT")
                for l in range(L):
                    x = layer(l, x, xT, lora_masks)
                    if l + 1 < L:
                        xT = transpose_to_fm(x, "xT")
                return x

            def cls_transpose(x_fin, tag):
                """extract CLS rows -> cls [NT, H] fp32; return nH x [P, NT] mdt."""
                cls_t = wk.tile([NT, H], f32, tag=f"cls_{tag}", bufs=1,
                                name=f"cls_{tag}")
                for s in range(NT):
                    nc.sync.dma_start(out=cls_t[s:s + 1, :], in_=x_fin[s][0:1, :])
                outs = []
                for c in range(nH):
                    ps = pp.tile([P, S], f32, tag="pattn", bufs=2, name=f"pscls{c}")
                    nc.tensor.transpose(ps[:, 0:NT], cls_t[0:NT, c * P:(c + 1) * P],
                                        ident_f[0:NT, 0:NT])
                    ct = wk.tile([P, NT], mdt, tag=f"clsT_{tag}", bufs=nH,
                                 name=f"clsT_{tag}{c}")
                    nc.scalar.copy(ct[:], ps[:, 0:NT])
                    outs.append(ct)
                return outs

            # ======================= PASS 1 =======================
            x_fin1 = encoder_pass(None)
            cls1T = cls_transpose(x_fin1, "p1")

            # router MLP
            rfc1_sb = []
            for c in range(nH):
                wt = wk.tile([P, H], mdt, tag="w_enc", bufs=nH + 4, name=f"rf1_{c}")
                nc.sync.dma_start(out=wt[:], in_=d["rfc1"][c * P:(c + 1) * P, :])
                rfc1_sb.append(wt)
            rfc2_sb = wk.tile([P, nH * T], mdt, tag="rf2", bufs=1, name="rf2")
            nc.sync.dma_start(out=rfc2_sb[:].rearrange("p (c t) -> p c t", t=T),
                              in_=d["rfc2"].rearrange("(c p) t -> p c t", p=P))
            z1T = []
            for mth in range(nH):
                ps = pp.tile([P, NTOK], f32, tag="ps512", bufs=2, name=f"psz{mth}")
                for c in range(nH):
                    nc.tensor.matmul(ps[:, 0:NT],
                                     lhsT=rfc1_sb[c][:, mth * P:(mth + 1) * P],
                                     rhs=cls1T[c][:], start=(c == 0),
                                     stop=(c == nH - 1))
                zt = wk.tile([P, NT], mdt, tag="z1T", bufs=nH, name=f"z1T{mth}")
                if flags["zb_r1"]:
                    nc.scalar.activation(zt[:], ps[:, 0:NT], AF.Relu)
                else:
                    bt = bias_col(d["rfc1_b"][mth * P:(mth + 1) * P, None], "bcol")
                    nc.scalar.activation(zt[:], ps[:, 0:NT], AF.Relu, bias=bt[:, :1])
                z1T.append(zt)

            psl = pp.tile([P, S], f32, tag="pattn", bufs=2, name="pslog")
            for c in range(nH):
                nc.tensor.matmul(psl[0:NT, 0:T], lhsT=z1T[c][:, 0:NT],
                                 rhs=rfc2_sb[:, c * T:(c + 1) * T],
                                 start=(c == 0), stop=(c == nH - 1))
            logits = wk.tile([NT, T], f32, tag="logits", bufs=1, name="logits")
            nc.vector.tensor_copy(logits[:], psl[0:NT, 0:T])
            if not flags["zb_r2"]:
                r2b = cpool.tile([NT, T], f32, name="r2b")
                nc.sync.dma_start(out=r2b[:],
                                  in_=d["rfc2_b"][None, :].to_broadcast([NT, T]))
                nc.vector.tensor_tensor(out=logits[:], in0=logits[:], in1=r2b[:],
                                        op=OP.add)
            nc.sync.dma_start(out=out_router[:, :], in_=logits[:])

            # one-hot of argmax + per-task token masks
            rmax = wk.tile([NT, 1], f32, tag="rmax", bufs=1, name="rmax")
            nc.vector.reduce_max(out=rmax[:], in_=logits[:], axis=X)
            onehot = wk.tile([NT, T], f32, tag="onehot", bufs=1, name="onehot")
            nc.vector.tensor_scalar(out=onehot[:], in0=logits[:],
                                    scalar1=rmax[:, :1], scalar2=None,
                                    op0=OP.is_ge)
            masks = []
            for t3 in range(T):
                psm = pp.tile([P, NTOK], f32, tag="ps512", bufs=2, name=f"psm{t3}")
                nc.tensor.matmul(psm[0:R, :],
                                 lhsT=onehot[:, t3:t3 + 1].to_broadcast([NT, R]),
                                 rhs=ssel[:], start=True, stop=True)
                mt = wk.tile([R, NTOK], f32, tag="masks", bufs=T, name=f"mask{t3}")
                nc.vector.tensor_copy(mt[:], psm[0:R, :])
                masks.append(mt)

            # ======================= PASS 2 =======================
            x_fin2 = encoder_pass(masks)
            cls2T = cls_transpose(x_fin2, "p2")

            # per-task heads, masked-combined
            blog = wk.tile([NT, 2], f32, tag="blog", bufs=1, name="blog")
            for t3 in range(T):
                wd_sb = []
                for c in range(nH):
                    wt = wk.tile([P, H], mdt, tag="w_enc", bufs=nH + 4,
                                 name=f"wd{t3}_{c}")
                    nc.sync.dma_start(out=wt[:], in_=d["wd"][t3, c * P:(c + 1) * P, :])
                    wd_sb.append(wt)
                wc_sb = wk.tile([P, nH * 2], mdt, tag="wc_sb", bufs=T, name=f"wc{t3}")
                nc.sync.dma_start(out=wc_sb[:].rearrange("p (c o) -> p c o", o=2),
                                  in_=d["wc"][t3].rearrange("(c p) o -> p c o", p=P))
                dT = []
                for mth in range(nH):
                    ps = pp.tile([P, NTOK], f32, tag="ps512", bufs=2,
                                 name=f"psd{t3}{mth}")
                    for c in range(nH):
                        nc.tensor.matmul(ps[:, 0:NT],
                                         lhsT=wd_sb[c][:, mth * P:(mth + 1) * P],
                                         rhs=cls2T[c][:], start=(c == 0),
                                         stop=(c == nH - 1))
                    dt_ = wk.tile([P, NT], mdt, tag="dT", bufs=nH, name=f"dT{t3}{mth}")
                    if flags["zb_d"]:
                        nc.scalar.activation(dt_[:], ps[:, 0:NT], AF.Tanh)
                    else:
                        bt = bias_col(d["bd"][t3, mth * P:(mth + 1) * P, None], "bcol")
                        nc.scalar.activation(dt_[:], ps[:, 0:NT], AF.Tanh,
                                             bias=bt[:, :1])
                    dT.append(dt_)
                psh = pp.tile([P, S], f32, tag="pattn", bufs=2, name=f"psht{t3}")
                for c in range(nH):
                    nc.tensor.matmul(psh[0:NT, 0:2], lhsT=dT[c][:, 0:NT],
                                     rhs=wc_sb[:, c * 2:(c + 1) * 2],
                                     start=(c == 0), stop=(c == nH - 1))
                lt = wk.tile([NT, 2], f32, tag="lt", bufs=2, name=f"lt{t3}")
                nc.vector.tensor_copy(lt[:], psh[0:NT, 0:2])
                if not flags["zb_c"]:
                    bcb = cpool.tile([NT, 2], f32, name=f"bcb{t3}")
                    nc.sync.dma_start(out=bcb[:],
                                      in_=d["bc"][t3, None, :].to_broadcast([NT, 2]))
                    nc.vector.tensor_tensor(out=lt[:], in0=lt[:], in1=bcb[:], op=OP.add)
                # mask by onehot[:, t3] and accumulate
                nc.vector.tensor_scalar(out=lt[:], in0=lt[:],
                                        scalar1=onehot[:, t3:t3 + 1], scalar2=None,
                                        op0=OP.mult)
                if t3 == 0:
                    nc.vector.tensor_copy(blog[:], lt[:])
                else:
                    nc.vector.tensor_tensor(out=blog[:], in0=blog[:], in1=lt[:],
                                            op=OP.add)
            nc.sync.dma_start(out=out_batch[:, :], in_=blog[:])

    return nc


# ----------------------------------------------------------------------------
# Public entry point
# ----------------------------------------------------------------------------
_CACHE = {}


def _in_maps_for(cfg, host, flags, inputs):
    b16 = ml_dtypes.bfloat16
    ids = np.asarray(inputs["input_ids"]).astype(np.int32)
    base = {
        "word_emb": host["word_emb"], "pos_emb": host["pos_emb"],
        "wq": host["wq"], "wk": host["wk"], "wv": host["wv"], "wo": host["wo"],
        "w1": host["w1"], "w2": host["w2"],
        "aqT": host["aqT"], "avT": host["avT"],
        "bqT": host["bqT"], "bvT": host["bvT"],
        "rfc1": host["rfc1"], "rfc2": host["rfc2"],
        "wd": host["wd"], "wc": host["wc"],
    }
    if not flags["zb_q"]: base["bq"] = host["bq"]
    if not flags["zb_k"]: base["bk"] = host["bk"]
    if not flags["zb_v"]: base["bv"] = host["bv"]
    if not flags["zb_o"]: base["bo"] = host["bo"]
    if not flags["zb_1"]: base["b1"] = host["b1"]
    if not flags["zb_2"]: base["b2"] = host["b2"]
    if not flags["zb_r1"]: base["rfc1_b"] = host["rfc1_b"]
    if not flags["zb_r2"]: base["rfc2_b"] = host["rfc2_b"]
    if not flags["zb_d"]: base["bd"] = host["bd"]
    if not flags["zb_c"]: base["bc"] = host["bc"]
    if not flags["id_emb_ln"]:
        base["emb_ln_g"] = host["emb_ln_g"]; base["emb_ln_b"] = host["emb_ln_b"]
    if not flags["id_ln1"]:
        base["ln1_g"] = host["ln1_g"]; base["ln1_b"] = host["ln1_b"]
    if not flags["id_ln2"]:
        base["ln2_g"] = host["ln2_g"]; base["ln2_b"] = host["ln2_b"]

    NT = cfg["B"] // cfg["n_cores"]
    in_maps = []
    for c in range(cfg["n_cores"]):
        m = dict(base)
        m["ids"] = np.ascontiguousarray(ids[c * NT:(c + 1) * NT])
        if not flags["mask_ones"]:
            am = np.asarray(inputs["attention_mask"]).astype(np.float32)
            bias = (am - 1.0) * np.float32(1e9 / np.sqrt(cfg["DH"]))
            m["attn_bias"] = np.ascontiguousarray(bias[c * NT:(c + 1) * NT])
        in_maps.append(m)
    return in_maps


def _ensure_axon_hooks():
    """Provide antenv.axon_hooks (missing in this image) so trace=True can
    capture NTFF profiles through the axon tunnel.  Mirrors
    trn_agent_boot.trn_boot._ntff_profile_via_ctypes."""
    import sys, types, ctypes, contextlib

    try:
        from antenv.axon_hooks import get_axon_ntff_profile_hook  # noqa
        return
    except ImportError:
        pass

    so_path = "/opt/axon/libaxon_pjrt.so"
    import os
    mod = types.ModuleType("antenv.axon_hooks")
    state = {"hook": None}

    def set_axon_ntff_profile_hook(h):
        state["hook"] = h

    def get_axon_ntff_profile_hook():
        return state["hook"]

    mod.set_axon_ntff_profile_hook = set_axon_ntff_profile_hook
    mod.get_axon_ntff_profile_hook = get_axon_ntff_profile_hook
    sys.modules["antenv.axon_hooks"] = mod
    import antenv
    antenv.axon_hooks = mod

    if not os.path.exists(so_path):
        return
    lib = ctypes.CDLL(so_path)
    if not hasattr(lib, "axon_start_nrt_profile"):
        return
    lib.axon_start_nrt_profile.argtypes = [ctypes.POINTER(ctypes.c_int64),
                                           ctypes.c_size_t]
    lib.axon_start_nrt_profile.restype = ctypes.c_int64
    lib.axon_stop_nrt_profile.argtypes = [ctypes.c_char_p]
    lib.axon_stop_nrt_profile.restype = ctypes.c_int64

    @contextlib.contextmanager
    def _hook(output_dir, device_ids):
        import jax
        jax.devices()
        if device_ids:
            ids = (ctypes.c_int64 * len(device_ids))(*device_ids)
            rc = lib.axon_start_nrt_profile(ids, len(device_ids))
        else:
            rc = lib.axon_start_nrt_profile(None, 0)
        if rc != 0:
            raise RuntimeError(f"axon_start_nrt_profile rc={rc}")
        try:
            yield
        finally:
            n = lib.axon_stop_nrt_profile(str(output_dir).encode())
            print(f"ntff profile: {n} file(s) written to {output_dir}",
                  file=sys.stderr)

    set_axon_ntff_profile_hook(_hook)


def run(inputs, cfg=None, mm_dt_name="bfloat16", trace=False):
    from concourse.bass_utils import run_bass_kernel_spmd
    if trace:
        _ensure_axon_hooks()

    cfg = dict(REAL_CFG if cfg is None else cfg)
    mm_np = ml_dtypes.bfloat16 if mm_dt_name == "bfloat16" else np.float32
    host, flags = _prep_host(inputs, cfg, mm_np)

    key = (tuple(sorted(cfg.items())), tuple(sorted(flags.items())), mm_dt_name)
    if key not in _CACHE:
        _CACHE[key] = _build(cfg, flags, mm_dt_name)
    nc = _CACHE[key]

    in_maps = _in_maps_for(cfg, host, flags, inputs)
    res = run_bass_kernel_spmd(nc, in_maps, core_ids=list(range(cfg["n_cores"])),
                               trace=trace)
    NT = cfg["B"] // cfg["n_cores"]
    rl = np.concatenate([r["router_logits"] for r in res.results], axis=0)
    bl = np.concatenate([r["batch_logits"] for r in res.results], axis=0)
    return (rl.astype(np.float32), bl.astype(np.float32)), res


def kernel(**inputs):
    (rl, bl), _ = run(inputs)
    return rl, bl
